# revision 1
# baseline (speedup 1.0000x reference)
"""8x8 blockwise 2D DCT on x[16,32,512,512] f32, data-parallel on 8 TRN2 cores.

Math: per 8x8 block B of the image, coeffs = D @ B @ D^T.  With
BD = blockdiag_16(D^T) [128,128], a [128h x 128w] chunk X satisfies:

  mm1: P1 = X^T @ BD    (contract h: column-DCT, output [w, h'])
  mm2: P2 = P1^T @ BD   (contract w: row-DCT,    output [h', w'])

Both matmuls use the chunk as the stationary operand (lhsT) and BD as the
moving operand, so each pass both applies the DCT and transposes -- after
two passes the data is back in its original orientation, no explicit
transposes needed.  mm1 runs in fp32 (input comes straight from DMA);
the PSUM->SBUF evacuation of P1 casts to bf16 so mm2 runs at full PE rate.

Sharding: pure data parallel along batch -- core i takes x[2i:2i+2],
flattened to [32768, 512] rows; each core does 256 slabs of [128, 512].
"""

import numpy as np

import concourse.bacc as bacc
import concourse.mybir as mybir
from concourse import tile
from concourse.bass_utils import run_bass_kernel_spmd

N_CORES = 8
B, C, H, W = 16, 32, 512, 512
ROWS_PER_CORE = (B // N_CORES) * C * H  # 32768
SLABS = ROWS_PER_CORE // 128            # 256

# knobs
NSLAB = 4          # slabs per DMA macro-tile (4 -> 1 MiB transfers)
MM1_BF16 = False   # False: mm1 in fp32 (no input cast); True: all-bf16
IN_BUFS = 3
OUT_BUFS = 3

_cached_nc = None


def _build_nc():
    f32 = mybir.dt.float32
    bf16 = mybir.dt.bfloat16
    nc = bacc.Bacc("TRN2", target_bir_lowering=False, debug=False,
                   num_devices=N_CORES)
    x_ext = nc.declare_dram_parameter("x", [ROWS_PER_CORE, W], f32,
                                      isOutput=False)
    bd_ext = nc.declare_dram_parameter("bd", [128, 128], f32, isOutput=False)
    out_ext = nc.declare_dram_parameter("out", [ROWS_PER_CORE, W], f32,
                                        isOutput=True)

    with tile.TileContext(nc) as tc:
        with (
            tc.tile_pool(name="const", bufs=1) as cpool,
            tc.tile_pool(name="xin", bufs=IN_BUFS) as xpool,
            tc.tile_pool(name="mid", bufs=4) as mpool,
            tc.tile_pool(name="oout", bufs=OUT_BUFS) as opool,
            tc.tile_pool(name="ps1p", bufs=3, space="PSUM") as ps1pool,
            tc.tile_pool(name="ps2p", bufs=3, space="PSUM") as ps2pool,
        ):
            bd32 = cpool.tile([128, 128], f32)
            nc.sync.dma_start(bd32[:], bd_ext[:, :])
            bd16 = cpool.tile([128, 128], bf16)
            nc.vector.tensor_copy(bd16[:], bd32[:])

            mm1_rhs = bd16 if MM1_BF16 else bd32

            for t in range(SLABS // NSLAB):
                r0 = t * NSLAB * 128
                xt = xpool.tile([128, NSLAB * W], f32, tag="xt")
                src = x_ext[r0:r0 + NSLAB * 128, :].rearrange(
                    "(n p) w -> p n w", p=128)
                nc.sync.dma_start(xt.rearrange("p (n w) -> p n w", n=NSLAB),
                                  src)
                if MM1_BF16:
                    xb = xpool.tile([128, NSLAB * W], bf16, tag="xb")
                    nc.scalar.copy(xb[:], xt[:])
                    mm1_in = xb
                else:
                    mm1_in = xt

                ot = opool.tile([128, NSLAB * W], f32, tag="ot")
                for n in range(NSLAB):
                    ps1 = ps1pool.tile([128, 512], f32, tag="ps1")
                    for c in range(4):
                        nc.tensor.matmul(
                            ps1[:, c * 128:(c + 1) * 128],
                            lhsT=mm1_in[:, n * W + c * 128:n * W + (c + 1) * 128],
                            rhs=mm1_rhs[:],
                            start=True, stop=True)
                    t1 = mpool.tile([128, 512], bf16, tag="t1")
                    nc.vector.tensor_copy(t1[:], ps1[:])
                    ps2 = ps2pool.tile([128, 512], f32, tag="ps2")
                    for c in range(4):
                        nc.tensor.matmul(
                            ps2[:, c * 128:(c + 1) * 128],
                            lhsT=t1[:, c * 128:(c + 1) * 128],
                            rhs=bd16[:],
                            start=True, stop=True)
                    nc.vector.tensor_copy(ot[:, n * W:(n + 1) * W], ps2[:])

                dst = out_ext[r0:r0 + NSLAB * 128, :].rearrange(
                    "(n p) w -> p n w", p=128)
                nc.scalar.dma_start(dst,
                                    ot.rearrange("p (n w) -> p n w", n=NSLAB))
    nc.compile()
    return nc


def _get_nc():
    global _cached_nc
    if _cached_nc is None:
        _cached_nc = _build_nc()
    return _cached_nc


def kernel(x, dct_matrix):
    x = np.asarray(x, dtype=np.float32)
    d = np.asarray(dct_matrix, dtype=np.float32)
    assert x.shape == (B, C, H, W), x.shape
    assert d.shape == (8, 8), d.shape

    bd = np.kron(np.eye(16, dtype=np.float32), d.T).astype(np.float32)
    flat = x.reshape(B * C * H, W)
    in_maps = [
        {"x": flat[i * ROWS_PER_CORE:(i + 1) * ROWS_PER_CORE], "bd": bd}
        for i in range(N_CORES)
    ]
    nc = _get_nc()
    res = run_bass_kernel_spmd(nc, in_maps, core_ids=list(range(N_CORES)))
    out = np.empty((B * C * H, W), dtype=np.float32)
    for i in range(N_CORES):
        out[i * ROWS_PER_CORE:(i + 1) * ROWS_PER_CORE] = res.results[i]["out"]
    return out.reshape(B, C, H, W)
